# revision 12
# baseline (speedup 1.0000x reference)
"""GCN layer (nn_GCNLayer_901943132166) on 8 Trainium2 NeuronCores — v3.

Strategy:
  - dst-partition nodes across 8 cores (12544/core = 25 PSUM super-blocks of
    512 dst, each 4 static windows of 128 dst).
  - Edges per core sorted by (sb, src-range, window, dst); chunks of 128
    edges, each chunk inside one (window, range) cell. Cell chunk counts are
    shared across cores (max over cores) so the program is SPMD; pad slots
    use idx=0 with S=0 (numerically inert).
  - Gather: dma_gather (SWDGE) of feature rows, int16 idxs per 32768-row src
    range, 896-idx calls round-robin across 4 SWDGE queues — the Q7 gather
    ucode runs on core-pair == queue, so 4 queues generate descriptors in
    parallel (~3.8x measured).
  - Aggregation: per chunk one bf16 matmul (lhsT=rows [128e,128f], rhs=S
    [128e,128] bf16) accumulating aggT[f, window] in PSUM at a static column
    offset. S = one-hot(dst - w*128) * (norm_src*norm_dst), host-built from
    index data. First chunk of each window writes with start=True (resets
    the region), so no separate PSUM init is needed.
  - Epilogue per sb: psT -> bf16 aggT (ACT copy), 4 matmuls vs W (bf16),
    bias add (DVE) -> bf16 out -> DRAM.
  - feat can ship as f32 (device casts per call, host touches only index
    metadata) or bf16 (FEAT_BF16=True, halves gather DMA bytes).
"""
import numpy as np
import ml_dtypes

BF16 = ml_dtypes.bfloat16
FP8 = ml_dtypes.float8_e4m3

N_NODES = 100000
N_EDGES = 1600000
F = 128
N_CORES = 8
OWN = 12544
SB = 512
NSB = 25
W = 256               # dst window width (static psum offsets)
WPS = SB // W         # windows per super-block
RANGE = 32768
NRANGES = 4
SCRATCH = 16384
MAX_CALL_CHUNKS = 7   # 896 idxs per call (hard SWDGE ring limit)
NQ = 4
FEAT_BF16 = True


def _install_walrus_passes():
    import concourse.bass_utils as bu

    def patched(tmpdir, inp="bir.json", outp="file.neff", arch=None, *, dve_root=None):
        from pathlib import Path
        cmd = [
            bu.get_walrus_driver(),
            "--pass",
            "birverifier,dynamic_dma_scan,runtime_memory_reservation,"
            "dynamic_dma_setup,lower_act,lower_dve,lower_ap_offset,"
            "codegen,neff_packager",
            "-i", inp,
            "--neff-output-filename", outp,
            "--enable-birsim=true",
            "--mem-mode=physical",
            "--policy=0",
            "--enable-ldw-opt=false",
            "--assign-static-dmas-to-sp=false",
            "--dram-page-size=256",
            "--enable-neff-debug-info=true",
            "--jobs", "8",
            f"--dynamic-dma-scratch-size-per-partition={SCRATCH}",
            *bu.get_walrus_args(
                bu.get_bir_arch(tmpdir, inp) if arch is None else arch,
                tmpdir, dve_root=dve_root,
            ),
        ]
        result = bu.run_command(cmd, cwd=tmpdir)
        if result is not None:
            (Path(tmpdir) / "log.txt").write_text(result.stdout)
        return f"{tmpdir}/{outp}"

    bu.bir_verify_and_optimise = patched


def _preprocess(src: np.ndarray, dst: np.ndarray):
    """Host-side index marshaling (indices and degree norms only).

    Returns (cellch [NSB,NRANGES,WPS], TC, per_core[(idx_buf, sdat)]).
    Chunk order: sb-major, then range, then window.
    """
    src = np.asarray(src).astype(np.int64)
    dst = np.asarray(dst).astype(np.int64)

    out_deg = np.bincount(src, minlength=N_NODES).astype(np.float32)
    in_deg = np.bincount(dst, minlength=N_NODES).astype(np.float32)
    norm_src = 1.0 / np.sqrt(np.clip(out_deg, 1.0, None))
    norm_dst = 1.0 / np.sqrt(np.clip(in_deg, 1.0, None))
    norm_edge = (norm_src[src] * norm_dst[dst]).astype(np.float32)

    core = np.minimum(dst // OWN, N_CORES - 1)
    dst_local = dst - core * OWN
    sb = dst_local // SB
    rng = src // RANGE
    win = (dst_local % SB) // W          # window within sb: 0..3
    col = dst_local % W                  # column within window

    # edges per (core, sb, range, window)
    cnt = np.zeros((N_CORES, NSB, NRANGES, WPS), np.int64)
    np.add.at(cnt, (core, sb, rng, win), 1)
    cellch = np.maximum((cnt.max(axis=0) + 127) // 128, 1)   # [NSB,NR,WPS]
    TC = int(cellch.sum())

    # global chunk index of each cell, in (sb, r, w) order
    cell_base = np.zeros((NSB, NRANGES, WPS), np.int64)
    acc = 0
    for s in range(NSB):
        for r in range(NRANGES):
            for w in range(WPS):
                cell_base[s, r, w] = acc
                acc += int(cellch[s, r, w])
    assert acc == TC

    per_core = []
    for k in range(N_CORES):
        m = core == k
        e_sb, e_rng, e_win = sb[m], rng[m], win[m]
        e_src, e_col, e_val = src[m], col[m], norm_edge[m]
        order = np.lexsort((e_col, e_win, e_rng, e_sb))
        e_sb, e_rng, e_win = e_sb[order], e_rng[order], e_win[order]
        e_src, e_col, e_val = e_src[order], e_col[order], e_val[order]

        # slot of each edge: cell_base[cell]*128 + position within cell
        ck = cnt[k]
        cstart = np.zeros((NSB, NRANGES, WPS), np.int64)
        acck = 0
        for s in range(NSB):
            for r in range(NRANGES):
                for w in range(WPS):
                    cstart[s, r, w] = acck
                    acck += int(ck[s, r, w])
        pos = np.arange(len(e_src)) - cstart[e_sb, e_rng, e_win]
        slot = cell_base[e_sb, e_rng, e_win] * 128 + pos

        idx_flat = np.zeros(TC * 128, np.int16)
        idx_flat[slot] = (e_src - e_rng * RANGE).astype(np.int16)
        sval = np.zeros((TC * 128,), np.float32)
        colv = np.zeros((TC * 128,), np.int64)
        sval[slot] = 1.0
        colv[slot] = e_col

        # idx wrap layout [128, TC*8]: slot j of chunk c -> [j%16, 16-tiled]
        iw = idx_flat.reshape(TC * 8, 16).T    # [16, TC*8]
        idx_buf = np.tile(iw, (8, 1)).astype(np.int16)

        # S: [128, TC*W] bf16: slot (c,p) value at column c*W + col
        p_of = np.arange(TC * 128) % 128
        c_of = np.arange(TC * 128) // 128
        sdat = np.zeros((128, TC * W), np.float32)
        sdat[p_of, c_of * W + colv] = sval
        per_core.append((idx_buf, sdat.astype(FP8)))

    return cellch, TC, per_core, norm_src, norm_dst


def _build_program(cellch, TC):
    import concourse.bacc as bacc
    import concourse.mybir as mybir
    import concourse.tile as tile

    nc = bacc.Bacc(num_swdge_queues=NQ, dynamic_dma_scratch_size=SCRATCH)
    feat_dt = mybir.dt.bfloat16 if FEAT_BF16 else mybir.dt.float32
    feat_d = nc.declare_dram_parameter("feat", [N_NODES, F], feat_dt, isOutput=False)
    w_d = nc.declare_dram_parameter("w", [F, F], mybir.dt.bfloat16, isOutput=False)
    bias_d = nc.declare_dram_parameter("biasb", [128, SB], mybir.dt.float32, isOutput=False)
    idx_d = nc.declare_dram_parameter("idxb", [128, TC * 8], mybir.dt.int16, isOutput=False)
    sdat_d = nc.declare_dram_parameter("sdat", [128, TC * W], mybir.dt.float8e4, isOutput=False)
    zc_d = nc.declare_dram_parameter("zc", [1, SB], mybir.dt.bfloat16, isOutput=False)
    ndst_d = nc.declare_dram_parameter("ndst", [128, NSB * (SB // F)], mybir.dt.float32, isOutput=False)
    out_d = nc.declare_dram_parameter("out", [NSB * SB, F], mybir.dt.bfloat16, isOutput=True)

    ranges = [(r * RANGE, min((r + 1) * RANGE, N_NODES)) for r in range(NRANGES)]

    with tile.TileContext(nc) as tc:
        with (
            tc.tile_pool(name="const", bufs=1) as constp,
            tc.tile_pool(name="et", bufs=3) as etp,
            tc.tile_pool(name="sd", bufs=3) as sdp,
            tc.tile_pool(name="agg", bufs=2) as aggp,
            tc.tile_pool(name="outs", bufs=2) as outsp,
            tc.tile_pool(name="ps", bufs=2, space="PSUM") as psp,
            tc.tile_pool(name="ps2", bufs=2, space="PSUM") as ps2p,
        ):
            w_t = constp.tile([F, F], mybir.dt.bfloat16)
            nc.sync.dma_start(w_t[:], w_d[:])
            bias_t = constp.tile([128, SB], mybir.dt.float32)
            nc.sync.dma_start(bias_t[:], bias_d[:])
            idx_t = constp.tile([128, TC * 8], mybir.dt.int16)
            nc.sync.dma_start(idx_t[:], idx_d[:])
            zc_t = constp.tile([1, SB], mybir.dt.bfloat16)
            nc.sync.dma_start(zc_t[:], zc_d[:])
            ndst_t = constp.tile([128, NSB * (SB // F)], mybir.dt.float32)
            nc.sync.dma_start(ndst_t[:], ndst_d[:])

            qctr = 0
            cbase = 0   # global chunk counter (must match cell_base order)
            for s in range(NSB):
                nch = int(cellch[s].sum())
                sb_c0 = cbase

                sd = sdp.tile([128, nch * W], mybir.dt.float8e4, tag="sd")
                nc.sync.dma_start(sd[:], sdat_d[:, sb_c0 * W: (sb_c0 + nch) * W])
                et = etp.tile([128, nch * F], mybir.dt.bfloat16, tag="et")

                # gathers: per range, calls of up to MAX_CALL_CHUNKS chunks
                for r in range(NRANGES):
                    lo, hi = ranges[r]
                    rch = int(cellch[s, r].sum())
                    r0 = cbase
                    off = 0
                    while off < rch:
                        t = min(MAX_CALL_CHUNKS, rch - off)
                        cc = r0 + off
                        nc.gpsimd.dma_gather(
                            out_ap=et[:, (cc - sb_c0) * F: (cc - sb_c0 + t) * F]
                            .rearrange("p (c e) -> p c e", e=F),
                            in_ap=feat_d[lo:hi, :],
                            idxs_ap=idx_t[:, cc * 8: (cc + t) * 8],
                            num_idxs=t * 128,
                            num_idxs_reg=t * 128,
                            elem_size=F,
                            queue_num=qctr % NQ,
                        )
                        qctr += 1
                        off += t
                    cbase += rch

                # aggregation matmuls; explicit zero-init of the whole bank
                psT = psp.tile([128, SB], mybir.dt.float32, space="PSUM")
                nc.tensor.matmul(
                    out=psT[:],
                    lhsT=zc_t[0:1, 0:128],
                    rhs=zc_t[0:1, :],
                    start=True, stop=False,
                    skip_group_check=True,
                )
                lc = 0
                for r in range(NRANGES):
                    for w in range(WPS):
                        n = int(cellch[s, r, w])
                        for c in range(n):
                            nc.tensor.matmul(
                                out=psT[:, w * W: (w + 1) * W],
                                lhsT=et[:, lc * F: (lc + 1) * F],
                                rhs=sd[:, lc * W: (lc + 1) * W],
                                start=False,
                                stop=(lc == nch - 1),
                                skip_group_check=True,
                            )
                            lc += 1
                assert lc == nch

                aggT = aggp.tile([128, SB], mybir.dt.bfloat16)
                nc.scalar.copy(aggT[:], psT[:])
                ps2 = ps2p.tile([128, SB], mybir.dt.float32, space="PSUM")
                for j in range(SB // F):
                    nc.tensor.matmul(
                        out=ps2[:, j * F: (j + 1) * F],
                        lhsT=aggT[:, j * F: (j + 1) * F],
                        rhs=w_t[:],
                        start=True,
                        stop=True,
                    )
                oti = outsp.tile([128, SB], mybir.dt.bfloat16, tag="oti")
                for j in range(SB // F):
                    nc.vector.tensor_scalar(
                        out=oti[:, j * F: (j + 1) * F],
                        in0=ps2[:, j * F: (j + 1) * F],
                        scalar1=ndst_t[:, s * (SB // F) + j: s * (SB // F) + j + 1],
                        scalar2=None,
                        op0=mybir.AluOpType.mult,
                    )
                ot = outsp.tile([128, SB], mybir.dt.bfloat16, tag="ot")
                nc.vector.tensor_tensor(
                    out=ot[:], in0=oti[:], in1=bias_t[:], op=mybir.AluOpType.add,
                )
                nc.sync.dma_start(
                    out_d[s * SB: (s + 1) * SB, :].rearrange("(j p) f -> p j f", p=128),
                    ot[:].rearrange("p (j f) -> p j f", f=F),
                )
            assert cbase == TC
    nc.finalize()
    return nc


def kernel(feat, weight, bias, src, dst):
    _install_walrus_passes()
    from concourse.bass_utils import run_bass_kernel_spmd

    feat_np = np.ascontiguousarray(np.asarray(feat, dtype=np.float32))
    w_bf = np.ascontiguousarray(np.asarray(weight, dtype=np.float32)).astype(BF16)
    bias = np.asarray(bias, dtype=np.float32)

    cellch, TC, per_core, norm_src, norm_dst = _preprocess(src, dst)
    feat_np = (feat_np * norm_src[:, None]).astype(BF16)
    nc = _build_program(cellch, TC)

    bias_b = np.broadcast_to(np.tile(bias, SB // F)[None, :], (128, SB)).copy()
    zc = np.zeros((1, SB), BF16)

    in_maps = []
    for k in range(N_CORES):
        idx_buf, sdat = per_core[k]
        nd = np.zeros((128, NSB * (SB // F)), np.float32)
        base = k * OWN
        for s_ in range(NSB):
            for j_ in range(SB // F):
                d0 = base + s_ * SB + j_ * F
                n_ = max(0, min(F, N_NODES - d0))
                if n_ > 0:
                    nd[:n_, s_ * (SB // F) + j_] = norm_dst[d0: d0 + n_]
        in_maps.append({
            "feat": feat_np, "w": w_bf, "biasb": bias_b,
            "idxb": idx_buf, "sdat": sdat, "zc": zc, "ndst": nd,
        })
    res = run_bass_kernel_spmd(nc, in_maps, list(range(N_CORES)))
    out = np.empty((N_CORES * OWN, F), np.float32)
    for k in range(N_CORES):
        out[k * OWN: (k + 1) * OWN] = res.results[k]["out"][:OWN].astype(np.float32)
    return out[:N_NODES]
